# revision 1
# baseline (speedup 1.0000x reference)
"""CNF GNN message-passing layer on 8 Trainium2 NeuronCores (Bass/Tile).

Strategy (edge/graph parallel, clause-owner sharding):
  - Core k owns clause range [k*CPC, (k+1)*CPC) and processes exactly the
    edges whose clause falls in its range (~1/8 of edges), for BOTH message
    passing directions.
  - Phase 1 (l2c): gather raw lit_feat rows per edge (hardware dma_gather,
    int16 indices -> lit table split into <=32768-row chunks), segment-sum
    into per-clause-window PSUM accumulators via one-hot membership matmuls,
    then apply mean + W_l2c/b_l2c projection + relu + the [cembs|clause_feat]
    @ W_c2l + b_c2l projection entirely on-chip, producing the local slice of
    the Wh_c2l message table.  (segment_mean commutes with the linear layer:
    mean(Wh[src]) = mean(feat[src]) @ W + has_deg * b.)
  - Phase 2 (c2l): gather Wh_c2l rows from the LOCAL table slice per edge,
    segment-sum into full-range lit windows -> partial table T_k, then a
    ReduceScatter(add) across the 8 cores sums partials and hands each core
    its lit slice, which is finalized with mean (1/deg scale) + relu.
  - Degrees / reciprocals are index-only preprocessing, computed on host.

All per-core variation is carried in input data (index streams, membership
rel-ids, aux rows) so one SPMD program serves all 8 cores.
"""
import sys
sys.path.insert(0, "/opt/trn_rl_repo")

import math
import numpy as np

P = 128           # partitions / tile edge
D = 128           # feature width (all of IN/CLAUSE/OUT sizes)
NCORES = 8
WG = 8            # windows per gather-group
CHUNK_MAX = 32768  # int16 index range for dma_gather
NQ = 4            # SWDGE queues for gather descriptor generation

F16 = True        # table/stream dtype: float16 (False -> float32)


# ----------------------------------------------------------------------------
# host-side graph preprocessing
# ----------------------------------------------------------------------------

def _ceil_to(x, m):
    return (x + m - 1) // m * m


def _chunk_bounds(nrows):
    """Split [0, nrows) into chunks of <= CHUNK_MAX rows."""
    nch = max(1, math.ceil(nrows / CHUNK_MAX))
    bounds = [min(i * CHUNK_MAX, nrows) for i in range(nch + 1)]
    bounds[-1] = nrows
    return bounds


def _build_streams(dst_local, src, vals, n_win, bounds, quotas, n_group):
    """Build gather-idx / membership-rel streams for one core & one phase.

    dst_local: per-edge destination-window-local id  (win*128 + rel)
    src:       per-edge source row id (into the gather table)
    vals:      optional per-edge scale (e.g. 1/deg of destination); scattered
               into a per-slot array (pads = 0)
    n_win:     number of 128-row destination windows (padded to n_group*WG)
    bounds:    chunk boundaries over the source-row space
    quotas:    tiles (128-slot groups) per (window, chunk)
    """
    nch = len(bounds) - 1
    ncols = 8 * sum(quotas) // 8  # columns per group block = sum over chunks of WG*q... computed below
    # staging column layout within a group block:
    #   chunk c occupies cols [colbase[c], colbase[c] + WG*quotas[c])
    #   window w (group-local), tile t -> col colbase[c] + w*quotas[c] + t
    colbase = []
    acc = 0
    for c in range(nch):
        colbase.append(acc)
        acc += WG * quotas[c]
    ncols = acc

    win = dst_local >> 7
    rel = dst_local & 127
    chunk = np.searchsorted(bounds, src, side="right") - 1
    # order edges by (window, chunk)
    key = win * nch + chunk
    order = np.argsort(key, kind="stable")
    key_s = key[order]
    src_s = src[order]
    rel_s = rel[order]
    chunk_s = chunk[order]
    win_s = win[order]
    # rank within each (window, chunk) run
    starts = np.searchsorted(key_s, np.arange(n_win * nch))
    run_start = starts[key_s]
    rank = np.arange(len(key_s)) - run_start
    counts = np.bincount(key_s, minlength=n_win * nch).reshape(n_win, nch)
    for c in range(nch):
        assert counts[:, c].max(initial=0) <= quotas[c] * P, (
            f"chunk {c} count {counts[:, c].max()} exceeds quota {quotas[c] * P}")

    # slot address: group g, col (within group block), partition p
    g = win_s // WG
    wl = win_s % WG
    col = np.array(colbase)[chunk_s] + wl * np.array(quotas)[chunk_s] + (rank >> 7)
    p = rank & 127
    flatcol = g * ncols + col

    n_group_cols = n_group * ncols
    rel_arr = np.full((P, n_group_cols), -1.0, np.float16 if F16 else np.float32)
    rel_arr[p, flatcol] = rel_s.astype(rel_arr.dtype)
    val_arr = None
    if vals is not None:
        val_arr = np.zeros((P, n_group_cols), rel_arr.dtype)
        val_arr[p, flatcol] = vals[order].astype(rel_arr.dtype)

    # idx streams per chunk: call for (group, chunk) covers WG*quotas[c]*128 slots,
    # enumerated col-major (slot i = col_local*128 + p)
    idx_streams = []
    for c in range(nch):
        qc = quotas[c]
        ncall = WG * qc * P               # idx per call
        arr = np.zeros((n_group, ncall), np.int16)
        m = chunk_s == c
        # call-local position: (wl*qc + tile)*128 + p  == (col - colbase[c])*128 + p
        pos = (col[m] - colbase[c]) * P + p[m]
        arr[g[m], pos] = (src_s[m] - bounds[c]).astype(np.int16)
        # wrap into 16 partitions, replicate x8 -> [128, n_group*ncall/16]
        w = arr.reshape(n_group, ncall // 16, 16).transpose(2, 0, 1).reshape(16, -1)
        idx_streams.append(np.tile(w, (8, 1)).copy())
    return idx_streams, rel_arr, val_arr, ncols, colbase


def _prep(inputs):
    """All host preprocessing. Returns (meta, in_maps)."""
    lit_feat = np.asarray(inputs["lit_feat"], np.float32)
    clause_feat = np.asarray(inputs["clause_feat"], np.float32)
    el = np.asarray(inputs["edge_lit"]).astype(np.int64)
    ec = np.asarray(inputs["edge_clause"]).astype(np.int64)
    W_l2c = np.asarray(inputs["W_l2c"], np.float32)
    b_l2c = np.asarray(inputs["b_l2c"], np.float32)
    W_c2l = np.asarray(inputs["W_c2l"], np.float32)
    b_c2l = np.asarray(inputs["b_c2l"], np.float32)

    n_lit = lit_feat.shape[0]
    n_clause = clause_feat.shape[0]
    tdt = np.float16 if F16 else np.float32

    CPC = n_clause // NCORES                       # clauses per core
    NWIN1 = _ceil_to(_ceil_to(CPC, P) // P, WG)    # clause windows per core (padded)
    NG1 = NWIN1 // WG
    CLROWS = NWIN1 * P                             # padded clause rows per core

    LITROWS = _ceil_to(n_lit, P)
    NWIN2 = _ceil_to(LITROWS // P, WG)             # lit windows (full range, padded)
    NG2 = NWIN2 // WG
    TROWS = NWIN2 * P                              # T table rows (div by 8*... )
    assert TROWS % NCORES == 0
    SLICE = TROWS // NCORES                        # rows per core post-RS
    NW3 = SLICE // P                               # finalize windows per core

    # degrees (global, index-only)
    degc = np.bincount(ec, minlength=n_clause).astype(np.float32)
    degl = np.bincount(el, minlength=n_lit).astype(np.float32)
    recipc = 1.0 / np.maximum(degc, 1.0)
    hasc = (degc > 0).astype(np.float32)

    owner = ec // CPC
    # phase-1 source chunking over lit rows
    b1 = _chunk_bounds(n_lit)
    # phase-2 source chunking over local clause table rows
    b2 = _chunk_bounds(CLROWS)

    # data-driven quotas (max over cores)
    lc = ec - owner * CPC                          # local clause id
    win1 = lc >> 7
    ch1 = np.searchsorted(b1, el, side="right") - 1
    cnt1 = np.bincount(((owner * NWIN1 + win1) * (len(b1) - 1) + ch1).astype(np.int64),
                       minlength=NCORES * NWIN1 * (len(b1) - 1))
    cnt1 = cnt1.reshape(NCORES, NWIN1, len(b1) - 1)
    q1 = [max(1, int(math.ceil(cnt1[:, :, c].max() / P))) for c in range(len(b1) - 1)]

    win2 = el >> 7
    ch2 = np.searchsorted(b2, lc, side="right") - 1
    cnt2 = np.bincount(((owner * NWIN2 + win2) * (len(b2) - 1) + ch2).astype(np.int64),
                       minlength=NCORES * NWIN2 * (len(b2) - 1))
    cnt2 = cnt2.reshape(NCORES, NWIN2, len(b2) - 1)
    q2 = [max(1, int(math.ceil(cnt2[:, :, c].max() / P))) for c in range(len(b2) - 1)]

    lit16 = np.ascontiguousarray(lit_feat.astype(tdt))

    # reduce-scatter split count: chunks overlap the collective with phase 2
    RSC = 1
    for cand in (7, 5, 4, 3, 2):
        if NW3 % cand == 0 and NG2 % cand == 0:
            RSC = cand
            break

    def _cat_groups(streams, ngroup):
        """Concat per-chunk idx streams group-block-wise into one array."""
        widths = [s.shape[1] // ngroup for s in streams]
        out = np.empty((P, ngroup * sum(widths)), streams[0].dtype)
        o = 0
        for g in range(ngroup):
            for s, w in zip(streams, widths):
                out[:, o:o + w] = s[:, g * w:(g + 1) * w]
                o += w
        return out

    def _interleave(a, b, ngroup):
        """Per-group [a_block | b_block] interleave of two [P, ngroup*w] arrays."""
        w = a.shape[1] // ngroup
        out = np.empty((P, ngroup * 2 * w), a.dtype)
        for g in range(ngroup):
            out[:, g * 2 * w:g * 2 * w + w] = a[:, g * w:(g + 1) * w]
            out[:, g * 2 * w + w:(g + 1) * 2 * w] = b[:, g * w:(g + 1) * w]
        return out

    in_maps = []
    meta = None
    for k in range(NCORES):
        m = owner == k
        elk, eck, lck = el[m], ec[m], lc[m]
        idx1, rel1, rcp1, ncols1, cb1 = _build_streams(
            lck, elk, recipc[eck], NWIN1, b1, q1, NG1)
        idx2, rel2, _, ncols2, cb2 = _build_streams(
            elk * 1, lck, None, NWIN2, b2, q2, NG2)

        # aux rows over this core's padded clause rows
        cl_ids = np.arange(CLROWS) + k * CPC
        valid = cl_ids < n_clause
        cl_ids = np.minimum(cl_ids, n_clause - 1)
        a_has = np.where(valid, hasc[cl_ids], 0.0).astype(tdt)[None, :]
        a_cf = np.where(valid, clause_feat[cl_ids, 0], 0.0)
        a_ones = valid.astype(np.float32)
        a_cf2 = np.stack([a_cf, a_ones]).astype(tdt)

        # finalize: per-partition recip over this core's interleaved lit slice
        CH, CHS = TROWS // RSC, SLICE // RSC
        w_all = np.arange(NW3)
        c_of_w = w_all // (NW3 // RSC)
        loc_of_w = w_all % (NW3 // RSC)
        base = c_of_w * CH + k * CHS + loc_of_w * P
        lit_ids = base[:, None] + np.arange(P)[None, :]     # [NW3, P]
        lvalid = lit_ids < n_lit
        lit_ids = np.minimum(lit_ids, n_lit - 1)
        rlit = np.where(lvalid, 1.0 / np.maximum(degl[lit_ids], 1.0), 1.0)
        rlit = rlit.astype(np.float32).T.copy()             # [128, NW3]

        iota_sb = np.broadcast_to(np.arange(P, dtype=tdt), (P, P)).copy()

        im = {
            "lit16": lit16,
            "idxc1": _cat_groups(idx1, NG1),
            "idxc2": _cat_groups(idx2, NG2),
            "relrcp1": _interleave(rel1, rcp1, NG1),
            "rel2": rel2,
            "auxhas": a_has, "auxcf2": a_cf2,
            "rlit": rlit, "iota": iota_sb,
            "wl2c": W_l2c.astype(tdt),
            "brow": b_l2c.astype(tdt)[None, :],
            "wc2l": W_c2l[:D].astype(tdt),
            "wb2": np.stack([W_c2l[D], b_c2l]).astype(tdt),
        }
        in_maps.append(im)
        if meta is None:
            meta = dict(
                n_lit=n_lit, n_clause=n_clause, CPC=CPC,
                NWIN1=NWIN1, NG1=NG1, CLROWS=CLROWS,
                NWIN2=NWIN2, NG2=NG2, TROWS=TROWS, SLICE=SLICE, NW3=NW3,
                b1=b1, b2=b2, q1=q1, q2=q2, RSC=RSC,
                ncols1=ncols1, cb1=cb1, ncols2=ncols2, cb2=cb2,
            )
    return meta, in_maps


# ----------------------------------------------------------------------------
# bass program
# ----------------------------------------------------------------------------

def _build_nc(meta, reps=1, skip_rs=False):
    import concourse.bass as bass
    import concourse.bacc as bacc
    import concourse.mybir as mybir
    import concourse.tile as tile

    tdt = mybir.dt.float16 if F16 else mybir.dt.float32
    f32 = mybir.dt.float32

    NG1, NWIN1, ncols1, cb1, q1 = meta["NG1"], meta["NWIN1"], meta["ncols1"], meta["cb1"], meta["q1"]
    NG2, NWIN2, ncols2, cb2, q2 = meta["NG2"], meta["NWIN2"], meta["ncols2"], meta["cb2"], meta["q2"]
    CLROWS, TROWS, SLICE, NW3 = meta["CLROWS"], meta["TROWS"], meta["SLICE"], meta["NW3"]
    RSC = meta["RSC"]
    b1, b2 = meta["b1"], meta["b2"]
    nch1, nch2 = len(b1) - 1, len(b2) - 1
    n_lit = meta["n_lit"]
    CW1, CW2 = ncols1 * 8, ncols2 * 8          # idx cols (int16) per group
    WPC = NW3 // RSC                           # finalize windows per RS chunk
    GPC2 = NG2 // RSC                          # phase-2 groups per RS chunk

    nc = bacc.Bacc("TRN2", target_bir_lowering=False, debug=False,
                   num_devices=NCORES, num_swdge_queues=NQ)

    lit16 = nc.declare_dram_parameter("lit16", [n_lit, D], tdt, isOutput=False)
    idxc1 = nc.declare_dram_parameter("idxc1", [P, NG1 * CW1], mybir.dt.int16, isOutput=False)
    idxc2 = nc.declare_dram_parameter("idxc2", [P, NG2 * CW2], mybir.dt.int16, isOutput=False)
    relrcp1 = nc.declare_dram_parameter("relrcp1", [P, NG1 * 2 * ncols1], tdt, isOutput=False)
    rel2 = nc.declare_dram_parameter("rel2", [P, NG2 * ncols2], tdt, isOutput=False)
    auxhas = nc.declare_dram_parameter("auxhas", [1, CLROWS], tdt, isOutput=False)
    auxcf2 = nc.declare_dram_parameter("auxcf2", [2, CLROWS], tdt, isOutput=False)
    rlit = nc.declare_dram_parameter("rlit", [P, NW3], f32, isOutput=False)
    iota_e = nc.declare_dram_parameter("iota", [P, P], tdt, isOutput=False)
    wl2c_e = nc.declare_dram_parameter("wl2c", [D, D], tdt, isOutput=False)
    brow_e = nc.declare_dram_parameter("brow", [1, D], tdt, isOutput=False)
    wc2l_e = nc.declare_dram_parameter("wc2l", [D, D], tdt, isOutput=False)
    wb2_e = nc.declare_dram_parameter("wb2", [2, D], tdt, isOutput=False)
    out_e = nc.declare_dram_parameter("out", [SLICE, D], f32, isOutput=True)

    wh_tbl = nc.dram_tensor("wh_tbl", [CLROWS, D], tdt)
    t_tbl = nc.dram_tensor("t_tbl", [TROWS, D], tdt)
    t_red = nc.dram_tensor("t_red", [SLICE, D], tdt)

    # Tile round-robins Pool DMAs over 8 DMASW sem lanes in emission order;
    # aligning queue_num with that rotation keeps each sem lane single-queue
    # (required: a DMA sem is locked to one SWDGE queue).
    pool_dma_count = [0]

    def _next_q():
        q = pool_dma_count[0] % NQ
        pool_dma_count[0] += 1
        return q

    with tile.TileContext(nc) as tc:
        with (
            tc.tile_pool(name="const", bufs=1) as cpool,
            tc.tile_pool(name="stage", bufs=3) as stage,
            tc.tile_pool(name="memb", bufs=3) as membp,
            tc.tile_pool(name="aux", bufs=3) as auxp,
            tc.tile_pool(name="small", bufs=4) as small,
            tc.tile_pool(name="psum", bufs=2, space="PSUM") as psum,
        ):
            iota_t = cpool.tile([P, P], tdt, tag="iota")
            nc.sync.dma_start(out=iota_t[:], in_=iota_e[:, :])
            wl2c_t = cpool.tile([D, D], tdt, tag="wl2c")
            nc.sync.dma_start(out=wl2c_t[:], in_=wl2c_e[:, :])
            brow_t = cpool.tile([1, D], tdt, tag="brow")
            nc.sync.dma_start(out=brow_t[:], in_=brow_e[:, :])
            wc2l_t = cpool.tile([D, D], tdt, tag="wc2l")
            nc.sync.dma_start(out=wc2l_t[:], in_=wc2l_e[:, :])
            wb2_t = cpool.tile([2, D], tdt, tag="wb2")
            nc.sync.dma_start(out=wb2_t[:], in_=wb2_e[:, :])
            rlit_t = cpool.tile([P, NW3], f32, tag="rlit")
            nc.sync.dma_start(out=rlit_t[:], in_=rlit[:, :])

            for rep in range(reps):
                # ---------------- phase 1 ----------------
                for g in range(NG1):
                    st = stage.tile([P, ncols1, D], tdt, tag="st1")
                    it = small.tile([P, CW1], mybir.dt.int16, tag="i1")
                    nc.sync.dma_start(out=it[:], in_=idxc1[:, g * CW1:(g + 1) * CW1])
                    for c in range(nch1):
                        tot = WG * q1[c]          # staging columns for this chunk
                        # HW desc-ring limit: <=1024 idx per dma_gather call
                        for o in range(0, tot, 8):
                            n = min(8, tot - o)
                            nc.gpsimd.dma_gather(
                                out_ap=st[:, cb1[c] + o:cb1[c] + o + n, :],
                                in_ap=lit16[b1[c]:b1[c + 1], :],
                                idxs_ap=it[:, (cb1[c] + o) * 8:(cb1[c] + o + n) * 8],
                                num_idxs=n * P,
                                num_idxs_reg=n * P,
                                elem_size=D,
                                queue_num=_next_q(),
                            )
                    rr = auxp.tile([P, 2 * ncols1], tdt, tag="rr1")
                    nc.sync.dma_start(
                        out=rr[:], in_=relrcp1[:, g * 2 * ncols1:(g + 1) * 2 * ncols1])
                    rl = rr[:, 0:ncols1]
                    rc = rr[:, ncols1:2 * ncols1]
                    mb = membp.tile([P, ncols1, P], tdt, tag="mb1")
                    nc.vector.tensor_tensor(
                        out=mb[:],
                        in0=iota_t[:, None, :].to_broadcast([P, ncols1, P]),
                        in1=rl.to_broadcast([P, ncols1, P]),
                        op=mybir.AluOpType.is_equal,
                    )
                    # fold 1/deg(clause) into the gathered messages (per slot)
                    nc.vector.tensor_tensor(
                        out=st[:],
                        in0=st[:],
                        in1=rc[:, :, None].to_broadcast([P, ncols1, D]),
                        op=mybir.AluOpType.mult,
                    )
                    ahas = auxp.tile([1, WG * P], tdt, tag="ahas")
                    nc.sync.dma_start(out=ahas[:], in_=auxhas[:, g * WG * P:(g + 1) * WG * P])
                    acf2 = auxp.tile([2, WG * P], tdt, tag="acf2")
                    nc.sync.dma_start(out=acf2[:], in_=auxcf2[:, g * WG * P:(g + 1) * WG * P])

                    whg = small.tile([P, WG, P], tdt, tag="whg")
                    for w in range(WG):
                        acc = psum.tile([P, P], f32, space="PSUM", tag="acc1")
                        cols = []
                        for c in range(nch1):
                            cols += [cb1[c] + w * q1[c] + t for t in range(q1[c])]
                        for i, col in enumerate(cols):
                            nc.tensor.matmul(
                                out=acc[:], lhsT=st[:, col, :], rhs=mb[:, col, :],
                                start=(i == 0), stop=(i == len(cols) - 1),
                            )
                        # acc already holds meanT (1/deg folded at gather)
                        meanT = small.tile([P, P], tdt, tag="meanT")
                        nc.vector.tensor_copy(out=meanT[:], in_=acc[:])
                        # cembsT = relu(W^T @ meanT + b x has)
                        p2t = psum.tile([P, P], f32, space="PSUM", tag="proj1")
                        nc.tensor.matmul(out=p2t[:], lhsT=wl2c_t[:], rhs=meanT[:],
                                         start=True, stop=False)
                        nc.tensor.matmul(out=p2t[:], lhsT=brow_t[:],
                                         rhs=ahas[:, w * P:(w + 1) * P],
                                         start=False, stop=True)
                        cembsT = small.tile([P, P], tdt, tag="cembsT")
                        nc.scalar.activation(out=cembsT[:], in_=p2t[:],
                                             func=mybir.ActivationFunctionType.Relu)
                        # wh = cembs @ Wc2l + cf x W[128] + 1 x b
                        p3t = psum.tile([P, P], f32, space="PSUM", tag="proj2")
                        nc.tensor.matmul(out=p3t[:], lhsT=cembsT[:], rhs=wc2l_t[:],
                                         start=True, stop=False)
                        nc.tensor.matmul(out=p3t[:], lhsT=acf2[:, w * P:(w + 1) * P],
                                         rhs=wb2_t[:], start=False, stop=True)
                        nc.scalar.copy(out=whg[:, w, :], in_=p3t[:])
                    row0 = g * WG * P
                    nc.scalar.dma_start(
                        out=wh_tbl[row0:row0 + WG * P, :].rearrange(
                            "(w p) f -> p w f", p=P),
                        in_=whg[:])

                # ---------------- phase 2 (+ interleaved RS/finalize) --------
                def rs_start(cidx):
                    CH, CHS = TROWS // RSC, SLICE // RSC
                    if not skip_rs:
                        nc.gpsimd.collective_compute(
                            "ReduceScatter",
                            mybir.AluOpType.add,
                            replica_groups=[list(range(NCORES))],
                            ins=[t_tbl[cidx * CH:(cidx + 1) * CH, :]],
                            outs=[t_red[cidx * CHS:(cidx + 1) * CHS, :]],
                        )

                def fin_chunk(cidx):
                    # scalar-engine DMAs: keeps the sync queue (group idx/rel
                    # loads) from stalling behind RS completion
                    CH, CHS = TROWS // RSC, SLICE // RSC
                    src_fin = t_tbl if skip_rs else t_red
                    fin = small.tile([P, WPC, P], tdt, tag="fin_in")
                    nc.scalar.dma_start(
                        out=fin[:],
                        in_=src_fin[cidx * CHS:(cidx + 1) * CHS, :].rearrange(
                            "(w p) f -> p w f", p=P))
                    og = small.tile([P, WPC, P], f32, tag="fin_out")
                    for w2 in range(WPC):
                        wabs = cidx * WPC + w2
                        nc.scalar.activation(out=og[:, w2, :], in_=fin[:, w2, :],
                                             func=mybir.ActivationFunctionType.Relu,
                                             scale=rlit_t[:, wabs:wabs + 1])
                    nc.scalar.dma_start(
                        out=out_e[cidx * CHS:(cidx + 1) * CHS, :].rearrange(
                            "(w p) f -> p w f", p=P),
                        in_=og[:])

                for g in range(NG2):
                    st = stage.tile([P, ncols2, D], tdt, tag="st2")
                    it = small.tile([P, CW2], mybir.dt.int16, tag="i2")
                    nc.sync.dma_start(out=it[:], in_=idxc2[:, g * CW2:(g + 1) * CW2])
                    for c in range(nch2):
                        tot = WG * q2[c]
                        for o in range(0, tot, 8):
                            n = min(8, tot - o)
                            nc.gpsimd.dma_gather(
                                out_ap=st[:, cb2[c] + o:cb2[c] + o + n, :],
                                in_ap=wh_tbl[b2[c]:b2[c + 1], :],
                                idxs_ap=it[:, (cb2[c] + o) * 8:(cb2[c] + o + n) * 8],
                                num_idxs=n * P,
                                num_idxs_reg=n * P,
                                elem_size=D,
                                queue_num=_next_q(),
                            )
                    rl = auxp.tile([P, ncols2], tdt, tag="rl2")
                    nc.sync.dma_start(out=rl[:], in_=rel2[:, g * ncols2:(g + 1) * ncols2])
                    mb = membp.tile([P, ncols2, P], tdt, tag="mb2")
                    nc.vector.tensor_tensor(
                        out=mb[:],
                        in0=iota_t[:, None, :].to_broadcast([P, ncols2, P]),
                        in1=rl.to_broadcast([P, ncols2, P]),
                        op=mybir.AluOpType.is_equal,
                    )
                    tg = small.tile([P, WG, P], tdt, tag="tg")
                    for w in range(WG):
                        acc = psum.tile([P, P], f32, space="PSUM", tag="acc2")
                        cols = []
                        for c in range(nch2):
                            cols += [cb2[c] + w * q2[c] + t for t in range(q2[c])]
                        for i, col in enumerate(cols):
                            nc.tensor.matmul(
                                out=acc[:], lhsT=mb[:, col, :], rhs=st[:, col, :],
                                start=(i == 0), stop=(i == len(cols) - 1),
                            )
                        nc.scalar.copy(out=tg[:, w, :], in_=acc[:])
                    row0 = g * WG * P
                    nc.scalar.dma_start(
                        out=t_tbl[row0:row0 + WG * P, :].rearrange(
                            "(w p) f -> p w f", p=P),
                        in_=tg[:])
                    if NG2 % RSC == 0 and (g + 1) % GPC2 == 0:
                        cidx = (g + 1) // GPC2 - 1
                        rs_start(cidx)
                        if cidx >= 1:
                            fin_chunk(cidx - 1)   # one chunk behind its RS
                if NG2 % RSC == 0:
                    fin_chunk(RSC - 1)
                else:
                    for cidx in range(RSC):
                        rs_start(cidx)
                        fin_chunk(cidx)

    nc.compile()
    return nc


# ----------------------------------------------------------------------------
# SPMD runner (jitted shard_map over the 8 NeuronCores, cached for reuse)
# ----------------------------------------------------------------------------

class SpmdRunner:
    def __init__(self, nc, n_cores):
        import jax
        import concourse.mybir as mybir
        from concourse.bass2jax import (
            _bass_exec_p, install_neuronx_cc_hook, partition_id_tensor)
        from jax.sharding import Mesh, PartitionSpec
        from jax.experimental.shard_map import shard_map

        install_neuronx_cc_hook()
        self.jax = jax
        self.n_cores = n_cores
        partition_name = nc.partition_id_tensor.name if nc.partition_id_tensor else None
        in_names, out_names, out_avals, zero_shapes = [], [], [], []
        for alloc in nc.m.functions[0].allocations:
            if not isinstance(alloc, mybir.MemoryLocationSet):
                continue
            name = alloc.memorylocations[0].name
            if alloc.kind == "ExternalInput":
                if name != partition_name:
                    in_names.append(name)
            elif alloc.kind == "ExternalOutput":
                out_names.append(name)
                shape = tuple(alloc.tensor_shape)
                dtype = mybir.dt.np(alloc.dtype)
                out_avals.append(jax.core.ShapedArray(shape, dtype))
                zero_shapes.append((shape, dtype))
        self.in_names, self.out_names = in_names, out_names
        self.out_avals, self.zero_shapes = out_avals, zero_shapes
        n_params, n_outs = len(in_names), len(out_avals)
        all_in_names = list(in_names) + list(out_names)
        if partition_name is not None:
            all_in_names.append(partition_name)

        def _body(*args):
            operands = list(args)
            if partition_name is not None:
                operands.append(partition_id_tensor())
            outs = _bass_exec_p.bind(
                *operands,
                out_avals=tuple(out_avals),
                in_names=tuple(all_in_names),
                out_names=tuple(out_names),
                lowering_input_output_aliases=(),
                sim_require_finite=True,
                sim_require_nnan=True,
                nc=nc,
            )
            return tuple(outs)

        devices = jax.devices()[:n_cores]
        self.mesh = Mesh(np.asarray(devices), ("core",))
        in_specs = (PartitionSpec("core"),) * (n_params + n_outs)
        out_specs = (PartitionSpec("core"),) * n_outs
        self.fn = jax.jit(
            shard_map(_body, mesh=self.mesh, in_specs=in_specs,
                      out_specs=out_specs, check_rep=False),
            keep_unused=True,
        )
        self._device_args = None
        self._pspec = PartitionSpec

    def put_inputs(self, in_maps):
        jax = self.jax
        n = self.n_cores
        sharding = jax.sharding.NamedSharding(self.mesh, self._pspec("core"))
        args = []
        for name in self.in_names:
            cat = np.concatenate([np.asarray(in_maps[c][name]) for c in range(n)], axis=0)
            args.append(jax.device_put(cat, sharding))
        for shape, dtype in self.zero_shapes:
            z = np.zeros((n * shape[0], *shape[1:]), dtype)
            args.append(jax.device_put(z, sharding))
        self._device_args = args
        jax.block_until_ready(args)

    def run(self):
        outs = self.fn(*self._device_args)
        self.jax.block_until_ready(outs)
        return outs

    def results(self, outs):
        n = self.n_cores
        res = []
        for c in range(n):
            d = {}
            for i, name in enumerate(self.out_names):
                shp = self.out_avals[i].shape
                d[name] = np.asarray(outs[i]).reshape(n, *shp)[c]
            res.append(d)
        return res


# ----------------------------------------------------------------------------
# public entry point
# ----------------------------------------------------------------------------

_CACHE = {}


def _get_runner(meta, reps):
    key = (tuple(sorted(meta.items(), key=lambda kv: repr(kv[0]))).__repr__(), reps)
    if key not in _CACHE:
        nc = _build_nc(meta, reps=reps)
        _CACHE[key] = SpmdRunner(nc, NCORES)
    return _CACHE[key]


def assemble(meta, res):
    """Reassemble per-core RS-chunked output slices into the full table."""
    RSC, TROWS, SLICE = meta["RSC"], meta["TROWS"], meta["SLICE"]
    CH, CHS = TROWS // RSC, SLICE // RSC
    full = np.empty((TROWS, D), np.float32)
    for c in range(RSC):
        for k in range(NCORES):
            full[c * CH + k * CHS:c * CH + (k + 1) * CHS] = \
                res[k]["out"][c * CHS:(c + 1) * CHS]
    return np.ascontiguousarray(full[:meta["n_lit"]])


def kernel(**inputs):
    meta, in_maps = _prep(inputs)
    r = _get_runner(meta, reps=1)
    r.put_inputs(in_maps)
    outs = r.run()
    res = r.results(outs)
    return assemble(meta, res)



# revision 39
# speedup vs baseline: 35.4682x; 35.4682x over previous
"""CNF GNN message-passing layer on 8 Trainium2 NeuronCores (Bass/Tile).

Strategy (edge/graph parallel, clause-owner sharding):
  - Core k owns clause range [k*CPC, (k+1)*CPC) and processes exactly the
    edges whose clause falls in its range (~1/8 of edges), for BOTH message
    passing directions.
  - Phase 1 (l2c): gather raw lit_feat rows per edge (hardware dma_gather,
    int16 indices -> lit table split into <=32768-row chunks), segment-sum
    into per-clause-window PSUM accumulators via one-hot membership matmuls,
    then apply mean + W_l2c/b_l2c projection + relu + the [cembs|clause_feat]
    @ W_c2l + b_c2l projection entirely on-chip, producing the local slice of
    the Wh_c2l message table.  (segment_mean commutes with the linear layer:
    mean(Wh[src]) = mean(feat[src]) @ W + has_deg * b.)
  - Phase 2 (c2l): gather Wh_c2l rows from the LOCAL table slice per edge,
    segment-sum into full-range lit windows -> partial table T_k, then a
    ReduceScatter(add) across the 8 cores sums partials and hands each core
    its lit slice, which is finalized with mean (1/deg scale) + relu.
  - Degrees / reciprocals are index-only preprocessing, computed on host.

All per-core variation is carried in input data (index streams, membership
rel-ids, aux rows) so one SPMD program serves all 8 cores.
"""
import sys
sys.path.insert(0, "/opt/trn_rl_repo")

import math
import numpy as np

P = 128           # partitions / tile edge
D = 128           # feature width (all of IN/CLAUSE/OUT sizes)
NCORES = 8
import os as _os
WG = int(_os.environ.get("KWG", "8"))  # windows per gather-group
CHUNK_MAX = 32768  # int16 index range for dma_gather
NQ = 4            # SWDGE queues for gather descriptor generation

F16 = True        # table/stream dtype: float16 (False -> float32)


# ----------------------------------------------------------------------------
# host-side graph preprocessing
# ----------------------------------------------------------------------------

def _ceil_to(x, m):
    return (x + m - 1) // m * m


def _chunk_bounds(nrows):
    """Split [0, nrows) into chunks of <= CHUNK_MAX rows."""
    nch = max(1, math.ceil(nrows / CHUNK_MAX))
    bounds = [min(i * CHUNK_MAX, nrows) for i in range(nch + 1)]
    bounds[-1] = nrows
    return bounds


def _build_streams(dst_local, src, vals, n_win, bounds, quotas, n_group):
    """Build gather-idx / membership-rel streams for one core & one phase.

    dst_local: per-edge destination-window-local id  (win*128 + rel)
    src:       per-edge source row id (into the gather table)
    vals:      optional per-edge scale (e.g. 1/deg of destination); scattered
               into a per-slot array (pads = 0)
    n_win:     number of 128-row destination windows (padded to n_group*WG)
    bounds:    chunk boundaries over the source-row space
    quotas:    tiles (128-slot groups) per (window, chunk)
    """
    nch = len(bounds) - 1
    ncols = 8 * sum(quotas) // 8  # columns per group block = sum over chunks of WG*q... computed below
    # staging column layout within a group block:
    #   chunk c occupies cols [colbase[c], colbase[c] + WG*quotas[c])
    #   window w (group-local), tile t -> col colbase[c] + w*quotas[c] + t
    colbase = []
    acc = 0
    for c in range(nch):
        colbase.append(acc)
        acc += WG * quotas[c]
    ncols = acc

    win = dst_local >> 7
    rel = dst_local & 127
    chunk = np.searchsorted(bounds, src, side="right") - 1
    # order edges by (window, chunk)
    key = win * nch + chunk
    order = np.argsort(key, kind="stable")
    key_s = key[order]
    src_s = src[order]
    rel_s = rel[order]
    chunk_s = chunk[order]
    win_s = win[order]
    # rank within each (window, chunk) run
    starts = np.searchsorted(key_s, np.arange(n_win * nch))
    run_start = starts[key_s]
    rank = np.arange(len(key_s)) - run_start
    counts = np.bincount(key_s, minlength=n_win * nch).reshape(n_win, nch)
    for c in range(nch):
        assert counts[:, c].max(initial=0) <= quotas[c] * P, (
            f"chunk {c} count {counts[:, c].max()} exceeds quota {quotas[c] * P}")

    # slot address: group g, col (within group block), partition p
    g = win_s // WG
    wl = win_s % WG
    col = np.array(colbase)[chunk_s] + wl * np.array(quotas)[chunk_s] + (rank >> 7)
    p = rank & 127
    flatcol = g * ncols + col

    n_group_cols = n_group * ncols
    rel_arr = np.full((P, n_group_cols), -1.0, np.float16 if F16 else np.float32)
    rel_arr[p, flatcol] = rel_s.astype(rel_arr.dtype)
    val_arr = None
    if vals is not None:
        val_arr = np.zeros((P, n_group_cols), rel_arr.dtype)
        val_arr[p, flatcol] = vals[order].astype(rel_arr.dtype)

    # idx streams per chunk: call for (group, chunk) covers WG*quotas[c]*128 slots,
    # enumerated col-major (slot i = col_local*128 + p)
    idx_streams = []
    for c in range(nch):
        qc = quotas[c]
        ncall = WG * qc * P               # idx per call
        # padded slots must NOT share one address: concurrent same-address
        # HBM reads serialize the SDMA engines (measured 5-12x slowdown).
        # Spread pads over distinct rows of the chunk instead.
        crows = bounds[c + 1] - bounds[c]
        spread = (np.arange(ncall, dtype=np.int64) % crows).astype(np.int16)
        arr = np.broadcast_to(spread, (n_group, ncall)).copy()
        m = chunk_s == c
        # call-local position: (wl*qc + tile)*128 + p  == (col - colbase[c])*128 + p
        pos = (col[m] - colbase[c]) * P + p[m]
        arr[g[m], pos] = (src_s[m] - bounds[c]).astype(np.int16)
        # wrap into 16 partitions, replicate x8 -> [128, n_group*ncall/16]
        w = arr.reshape(n_group, ncall // 16, 16).transpose(2, 0, 1).reshape(16, -1)
        idx_streams.append(np.tile(w, (8, 1)).copy())
    return idx_streams, rel_arr, val_arr, ncols, colbase


def _prep(inputs):
    """All host preprocessing. Returns (meta, in_maps)."""
    lit_feat = np.asarray(inputs["lit_feat"], np.float32)
    clause_feat = np.asarray(inputs["clause_feat"], np.float32)
    el = np.asarray(inputs["edge_lit"]).astype(np.int64)
    ec = np.asarray(inputs["edge_clause"]).astype(np.int64)
    W_l2c = np.asarray(inputs["W_l2c"], np.float32)
    b_l2c = np.asarray(inputs["b_l2c"], np.float32)
    W_c2l = np.asarray(inputs["W_c2l"], np.float32)
    b_c2l = np.asarray(inputs["b_c2l"], np.float32)

    n_lit = lit_feat.shape[0]
    n_clause = clause_feat.shape[0]
    tdt = np.float16 if F16 else np.float32

    CPC = n_clause // NCORES                       # clauses per core
    NWIN1 = _ceil_to(_ceil_to(CPC, P) // P, WG)    # clause windows per core (padded)
    NG1 = NWIN1 // WG
    CLROWS = NWIN1 * P                             # padded clause rows per core

    LITROWS = _ceil_to(n_lit, P)
    NWIN2 = _ceil_to(LITROWS // P, WG)             # lit windows (full range, padded)
    NG2 = NWIN2 // WG
    TROWS = NWIN2 * P                              # T table rows (div by 8*... )
    assert TROWS % NCORES == 0
    SLICE = TROWS // NCORES                        # rows per core post-RS
    NW3 = SLICE // P                               # finalize windows per core

    # degrees (global, index-only)
    degc = np.bincount(ec, minlength=n_clause).astype(np.float32)
    degl = np.bincount(el, minlength=n_lit).astype(np.float32)
    recipc = 1.0 / np.maximum(degc, 1.0)
    hasc = (degc > 0).astype(np.float32)

    owner = ec // CPC
    # phase-1 source chunking over lit rows
    b1 = _chunk_bounds(n_lit)
    # phase-2 source chunking over local clause table rows
    b2 = _chunk_bounds(CLROWS)

    # data-driven quotas (max over cores)
    lc = ec - owner * CPC                          # local clause id
    win1 = lc >> 7
    ch1 = np.searchsorted(b1, el, side="right") - 1
    cnt1 = np.bincount(((owner * NWIN1 + win1) * (len(b1) - 1) + ch1).astype(np.int64),
                       minlength=NCORES * NWIN1 * (len(b1) - 1))
    cnt1 = cnt1.reshape(NCORES, NWIN1, len(b1) - 1)
    q1 = [max(1, int(math.ceil(cnt1[:, :, c].max() / P))) for c in range(len(b1) - 1)]

    win2 = el >> 7
    ch2 = np.searchsorted(b2, lc, side="right") - 1
    cnt2 = np.bincount(((owner * NWIN2 + win2) * (len(b2) - 1) + ch2).astype(np.int64),
                       minlength=NCORES * NWIN2 * (len(b2) - 1))
    cnt2 = cnt2.reshape(NCORES, NWIN2, len(b2) - 1)
    q2 = [max(1, int(math.ceil(cnt2[:, :, c].max() / P))) for c in range(len(b2) - 1)]

    lit16 = np.ascontiguousarray(lit_feat.astype(tdt))

    # reduce-scatter split count: chunks overlap the collective with phase 2
    RSC = 1
    for cand in (14, 7, 5, 4, 3, 2):
        if NW3 % cand == 0 and NG2 % cand == 0:
            RSC = cand
            break

    def _cat_groups(streams, ngroup):
        """Concat per-chunk idx streams group-block-wise into one array."""
        widths = [s.shape[1] // ngroup for s in streams]
        out = np.empty((P, ngroup * sum(widths)), streams[0].dtype)
        o = 0
        for g in range(ngroup):
            for s, w in zip(streams, widths):
                out[:, o:o + w] = s[:, g * w:(g + 1) * w]
                o += w
        return out

    def _interleave(a, b, ngroup):
        """Per-group [a_block | b_block] interleave of two [P, ngroup*w] arrays."""
        w = a.shape[1] // ngroup
        out = np.empty((P, ngroup * 2 * w), a.dtype)
        for g in range(ngroup):
            out[:, g * 2 * w:g * 2 * w + w] = a[:, g * w:(g + 1) * w]
            out[:, g * 2 * w + w:(g + 1) * 2 * w] = b[:, g * w:(g + 1) * w]
        return out

    in_maps = []
    meta = None
    for k in range(NCORES):
        m = owner == k
        elk, eck, lck = el[m], ec[m], lc[m]
        idx1, rel1, rcp1, ncols1, cb1 = _build_streams(
            lck, elk, recipc[eck], NWIN1, b1, q1, NG1)
        idx2, rel2, _, ncols2, cb2 = _build_streams(
            elk * 1, lck, None, NWIN2, b2, q2, NG2)

        # aux rows over this core's padded clause rows
        cl_ids = np.arange(CLROWS) + k * CPC
        valid = cl_ids < n_clause
        cl_ids = np.minimum(cl_ids, n_clause - 1)
        a_has = np.where(valid, hasc[cl_ids], 0.0).astype(tdt)[None, :]
        a_rcw = np.where(valid, recipc[cl_ids], 0.0).astype(tdt)[None, :]
        a_hr = np.concatenate([a_has, a_rcw], axis=0)          # [2, CLROWS]
        a_cf = np.where(valid, clause_feat[cl_ids, 0], 0.0)
        a_ones = valid.astype(np.float32)
        a_cf2 = np.stack([a_cf, a_ones]).astype(tdt)

        # finalize: per-partition recip over this core's interleaved lit slice
        CH, CHS = TROWS // RSC, SLICE // RSC
        w_all = np.arange(NW3)
        c_of_w = w_all // (NW3 // RSC)
        loc_of_w = w_all % (NW3 // RSC)
        base = c_of_w * CH + k * CHS + loc_of_w * P
        lit_ids = base[:, None] + np.arange(P)[None, :]     # [NW3, P]
        lvalid = lit_ids < n_lit
        lit_ids = np.minimum(lit_ids, n_lit - 1)
        rlit = np.where(lvalid, 1.0 / np.maximum(degl[lit_ids], 1.0), 1.0)
        rlit = rlit.astype(np.float32).T.copy()             # [128, NW3]

        iota_sb = np.broadcast_to(np.arange(P, dtype=tdt), (P, P)).copy()

        im = {
            "lit16": lit16,
            "idxc1": _cat_groups(idx1, NG1),
            "idxc2": _cat_groups(idx2, NG2),
            "rel1": _interleave(rel1, rcp1, NG1),
            "rel2": rel2,
            "auxhr": a_hr, "auxcf2": a_cf2,
            "rlit": rlit, "iota": iota_sb,
            "wl2c": W_l2c.astype(tdt),
            "brow": b_l2c.astype(tdt)[None, :],
            "wc2l": W_c2l[:D].astype(tdt),
            "wb2": np.stack([W_c2l[D], b_c2l]).astype(tdt),
        }
        in_maps.append(im)
        if meta is None:
            meta = dict(
                n_lit=n_lit, n_clause=n_clause, CPC=CPC,
                NWIN1=NWIN1, NG1=NG1, CLROWS=CLROWS,
                NWIN2=NWIN2, NG2=NG2, TROWS=TROWS, SLICE=SLICE, NW3=NW3,
                b1=b1, b2=b2, q1=q1, q2=q2, RSC=RSC,
                ncols1=ncols1, cb1=cb1, ncols2=ncols2, cb2=cb2,
            )
    return meta, in_maps


# ----------------------------------------------------------------------------
# bass program
# ----------------------------------------------------------------------------

def _build_nc(meta, reps=1, skip_rs=False, ablate=()):
    import concourse.bass as bass
    import concourse.bacc as bacc
    import concourse.mybir as mybir
    import concourse.tile as tile

    tdt = mybir.dt.float16 if F16 else mybir.dt.float32
    f32 = mybir.dt.float32

    NG1, NWIN1, ncols1, cb1, q1 = meta["NG1"], meta["NWIN1"], meta["ncols1"], meta["cb1"], meta["q1"]
    NG2, NWIN2, ncols2, cb2, q2 = meta["NG2"], meta["NWIN2"], meta["ncols2"], meta["cb2"], meta["q2"]
    CLROWS, TROWS, SLICE, NW3 = meta["CLROWS"], meta["TROWS"], meta["SLICE"], meta["NW3"]
    RSC = meta["RSC"]
    b1, b2 = meta["b1"], meta["b2"]
    nch1, nch2 = len(b1) - 1, len(b2) - 1
    n_lit = meta["n_lit"]
    CW1, CW2 = ncols1 * 8, ncols2 * 8          # idx cols (int16) per group
    WPC = NW3 // RSC                           # finalize windows per RS chunk
    GPC2 = NG2 // RSC                          # phase-2 groups per RS chunk
    RSD = 2                                    # groups of delay before rs launch
    FLG = GPC2                                 # extra lag before fin consumes RS

    nc = bacc.Bacc("TRN2", target_bir_lowering=False, debug=False,
                   num_devices=NCORES, num_swdge_queues=NQ)

    lit16 = nc.declare_dram_parameter("lit16", [n_lit, D], tdt, isOutput=False)
    idxc1 = nc.declare_dram_parameter("idxc1", [P, NG1 * CW1], mybir.dt.int16, isOutput=False)
    idxc2 = nc.declare_dram_parameter("idxc2", [P, NG2 * CW2], mybir.dt.int16, isOutput=False)
    rel1 = nc.declare_dram_parameter("rel1", [P, NG1 * 2 * ncols1], tdt, isOutput=False)
    rel2 = nc.declare_dram_parameter("rel2", [P, NG2 * ncols2], tdt, isOutput=False)
    auxhr = nc.declare_dram_parameter("auxhr", [2, CLROWS], tdt, isOutput=False)
    auxcf2 = nc.declare_dram_parameter("auxcf2", [2, CLROWS], tdt, isOutput=False)
    rlit = nc.declare_dram_parameter("rlit", [P, NW3], f32, isOutput=False)
    iota_e = nc.declare_dram_parameter("iota", [P, P], tdt, isOutput=False)
    wl2c_e = nc.declare_dram_parameter("wl2c", [D, D], tdt, isOutput=False)
    brow_e = nc.declare_dram_parameter("brow", [1, D], tdt, isOutput=False)
    wc2l_e = nc.declare_dram_parameter("wc2l", [D, D], tdt, isOutput=False)
    wb2_e = nc.declare_dram_parameter("wb2", [2, D], tdt, isOutput=False)
    out_e = nc.declare_dram_parameter("out", [SLICE, D], f32, isOutput=True)

    wh_tbl = nc.dram_tensor("wh_tbl", [CLROWS, D], tdt)
    t_tbl = nc.dram_tensor("t_tbl", [TROWS, D], tdt)
    t_red = nc.dram_tensor("t_red", [SLICE, D], tdt)
    junk1 = junk2 = None
    if "gather" in ablate:
        junk1 = nc.dram_tensor("junk1", [P, ncols1 * D], tdt)
        junk2 = nc.dram_tensor("junk2", [P, ncols2 * D], tdt)

    # Tile round-robins Pool DMAs over 8 DMASW sem lanes in emission order;
    # aligning queue_num with that rotation keeps each sem lane single-queue
    # (required: a DMA sem is locked to one SWDGE queue).
    pool_dma_count = [0]

    def _next_q():
        q = pool_dma_count[0] % NQ
        pool_dma_count[0] += 1
        return q

    with tile.TileContext(nc) as tc:
        import os
        _bs = int(os.environ.get("KBUFS", "3"))
        _bp = int(os.environ.get("KPSUM", "2"))
        with (
            tc.tile_pool(name="const", bufs=1) as cpool,
            tc.tile_pool(name="stage", bufs=_bs) as stage,
            tc.tile_pool(name="memb", bufs=_bs) as membp,
            tc.tile_pool(name="aux", bufs=_bs + 1) as auxp,
            tc.tile_pool(name="small", bufs=_bs + 1) as small,
            tc.tile_pool(name="psum", bufs=_bp, space="PSUM") as psum,
        ):
            iota_t = cpool.tile([P, P], tdt, tag="iota")
            nc.sync.dma_start(out=iota_t[:], in_=iota_e[:, :])
            wl2c_t = cpool.tile([D, D], tdt, tag="wl2c")
            nc.sync.dma_start(out=wl2c_t[:], in_=wl2c_e[:, :])
            brow_t = cpool.tile([1, D], tdt, tag="brow")
            nc.sync.dma_start(out=brow_t[:], in_=brow_e[:, :])
            wc2l_t = cpool.tile([D, D], tdt, tag="wc2l")
            nc.sync.dma_start(out=wc2l_t[:], in_=wc2l_e[:, :])
            wb2_t = cpool.tile([2, D], tdt, tag="wb2")
            nc.sync.dma_start(out=wb2_t[:], in_=wb2_e[:, :])
            rlit_t = cpool.tile([P, NW3], f32, tag="rlit")
            nc.sync.dma_start(out=rlit_t[:], in_=rlit[:, :])
            # one shared register for the (constant) gather call size instead
            # of a RegisterMove in front of every dma_gather on the Pool queue
            reg1024 = nc.gpsimd.to_reg(8 * P)

            for rep in range(reps):
                # ---------------- phase 1 ----------------
                for g in range(NG1):
                    if "phase1" in ablate:
                        break
                    st = stage.tile([P, ncols1, D], tdt, tag="st1")
                    it = small.tile([P, CW1], mybir.dt.int16, tag="i1")
                    nc.sync.dma_start(out=it[:], in_=idxc1[:, g * CW1:(g + 1) * CW1])
                    if "gather" in ablate:
                        nc.sync.dma_start(
                            out=st[:].rearrange("p c d -> p (c d)"),
                            in_=junk1[:, :])
                    for c in range(nch1):
                        if "gather" in ablate:
                            break
                        tot = WG * q1[c]          # staging columns for this chunk
                        # HW desc-ring limit: <=1024 idx per dma_gather call
                        for o in range(0, tot, 8):
                            n = min(8, tot - o)
                            nc.gpsimd.dma_gather(
                                out_ap=st[:, cb1[c] + o:cb1[c] + o + n, :],
                                in_ap=lit16[b1[c]:b1[c + 1], :],
                                idxs_ap=it[:, (cb1[c] + o) * 8:(cb1[c] + o + n) * 8],
                                num_idxs=n * P,
                                num_idxs_reg=reg1024 if n == 8 else n * P,
                                elem_size=D,
                                queue_num=_next_q(),
                            )
                    rr = auxp.tile([P, 2 * ncols1], tdt, tag="rr1")
                    nc.sync.dma_start(
                        out=rr[:], in_=rel1[:, g * 2 * ncols1:(g + 1) * 2 * ncols1])
                    rl = rr[:, 0:ncols1]
                    rc = rr[:, ncols1:2 * ncols1]
                    mb = None
                    if "memb" not in ablate:
                        # scaled one-hot: mb[e, c] = 1/deg(dst) * [rel[e] == c].
                        # Built from streams only (no dependence on gathered
                        # st), so DVE never waits on gather completion.
                        mb = membp.tile([P, ncols1, P], tdt, tag="mb1")
                        nc.vector.tensor_tensor(
                            out=mb[:],
                            in0=iota_t[:, None, :].to_broadcast([P, ncols1, P]),
                            in1=rl.to_broadcast([P, ncols1, P]),
                            op=mybir.AluOpType.is_equal,
                        )
                        nc.vector.tensor_tensor(
                            out=mb[:],
                            in0=mb[:],
                            in1=rc[:, :, None].to_broadcast([P, ncols1, P]),
                            op=mybir.AluOpType.mult,
                        )
                    ahr = auxp.tile([2, WG * P], tdt, tag="ahr")
                    nc.sync.dma_start(out=ahr[:], in_=auxhr[:, g * WG * P:(g + 1) * WG * P])
                    ahas = ahr[0:1, :]
                    acf2 = auxp.tile([2, WG * P], tdt, tag="acf2")
                    nc.sync.dma_start(out=acf2[:], in_=auxcf2[:, g * WG * P:(g + 1) * WG * P])

                    if "nomm" in ablate:
                        continue
                    mb_src = st if "memb" in ablate else mb
                    # 4 windows' segment-sums packed per PSUM bank tile; one
                    # DVE copy per quad amortizes the PSUM->SBUF drain.
                    meanA = small.tile([P, WG * P], tdt, tag="meanA")
                    for half in range(WG // 4):
                        accq = psum.tile([P, 4 * P], f32, space="PSUM",
                                         tag="acc", bufs=2)
                        for wq in range(4):
                            w = half * 4 + wq
                            cols = []
                            for c in range(nch1):
                                cols += [cb1[c] + w * q1[c] + t for t in range(q1[c])]
                            if "accmm" in ablate:
                                cols = cols[:1]
                            for i, col in enumerate(cols):
                                nc.tensor.matmul(
                                    out=accq[:, wq * P:(wq + 1) * P],
                                    lhsT=st[:, col, :],
                                    rhs=mb_src[:, col, 0:P],
                                    start=(i == 0), stop=(i == len(cols) - 1),
                                )
                        nc.vector.tensor_copy(
                            out=meanA[:, half * 4 * P:(half + 1) * 4 * P],
                            in_=accq[:])
                    whg = small.tile([P, WG, P], tdt, tag="whg")
                    whgf = whg[:].rearrange("p w f -> p (w f)")
                    if "proj" in ablate:
                        nc.scalar.copy(out=whgf, in_=meanA[:])
                    else:
                        # cembs batched 4 windows per 512-wide projection
                        # (one PSUM bank per matmul output)
                        cembsA = small.tile([P, WG * P], tdt, tag="cembsA")
                        for half in range(WG // 4):
                            sl = slice(half * 4 * P, (half + 1) * 4 * P)
                            p2q = psum.tile([P, 4 * P], f32, space="PSUM",
                                            tag="proj1", bufs=2)
                            nc.tensor.matmul(out=p2q[:], lhsT=wl2c_t[:],
                                             rhs=meanA[:, sl],
                                             start=True, stop=False)
                            nc.tensor.matmul(out=p2q[:], lhsT=brow_t[:],
                                             rhs=ahas[:, sl],
                                             start=False, stop=True)
                            nc.scalar.activation(out=cembsA[:, sl], in_=p2q[:],
                                                 func=mybir.ActivationFunctionType.Relu)
                        # wh = cembs @ Wc2l + cf x W[128] + 1 x b (per window:
                        # output rows = clauses must sit on partitions)
                        for w in range(WG):
                            p3t = psum.tile([P, P], f32, space="PSUM",
                                            tag="proj2", bufs=2)
                            nc.tensor.matmul(out=p3t[:],
                                             lhsT=cembsA[:, w * P:(w + 1) * P],
                                             rhs=wc2l_t[:], start=True, stop=False)
                            nc.tensor.matmul(out=p3t[:],
                                             lhsT=acf2[:, w * P:(w + 1) * P],
                                             rhs=wb2_t[:], start=False, stop=True)
                            nc.scalar.copy(out=whg[:, w, :], in_=p3t[:])
                    row0 = g * WG * P
                    if "tblwrite" not in ablate:
                        nc.scalar.dma_start(
                            out=wh_tbl[row0:row0 + WG * P, :].rearrange(
                                "(w p) f -> p w f", p=P),
                            in_=whg[:])

                # ---------------- phase 2 (+ interleaved RS/finalize) --------
                def rs_start(cidx):
                    CH, CHS = TROWS // RSC, SLICE // RSC
                    if not skip_rs and "rs" not in ablate:
                        nc.gpsimd.collective_compute(
                            "ReduceScatter",
                            mybir.AluOpType.add,
                            replica_groups=[list(range(NCORES))],
                            ins=[t_tbl[cidx * CH:(cidx + 1) * CH, :]],
                            outs=[t_red[cidx * CHS:(cidx + 1) * CHS, :]],
                        )

                def fin_chunk(cidx):
                    if "fin" in ablate:
                        return
                    # scalar-engine DMAs: keeps the sync queue (group idx/rel
                    # loads) from stalling behind RS completion
                    CH, CHS = TROWS // RSC, SLICE // RSC
                    src_fin = t_tbl if (skip_rs or "rs" in ablate) else t_red
                    fin = small.tile([P, WPC, P], tdt, tag="fin_in")
                    nc.scalar.dma_start(
                        out=fin[:],
                        in_=src_fin[cidx * CHS:(cidx + 1) * CHS, :].rearrange(
                            "(w p) f -> p w f", p=P))
                    og = small.tile([P, WPC, P], f32, tag="fin_out")
                    for w2 in range(WPC):
                        wabs = cidx * WPC + w2
                        nc.scalar.activation(out=og[:, w2, :], in_=fin[:, w2, :],
                                             func=mybir.ActivationFunctionType.Relu,
                                             scale=rlit_t[:, wabs:wabs + 1])
                    nc.scalar.dma_start(
                        out=out_e[cidx * CHS:(cidx + 1) * CHS, :].rearrange(
                            "(w p) f -> p w f", p=P),
                        in_=og[:])

                for g in range(NG2):
                    if "phase2" in ablate:
                        break
                    st = stage.tile([P, ncols2, D], tdt, tag="st2")
                    it = small.tile([P, CW2], mybir.dt.int16, tag="i2")
                    nc.sync.dma_start(out=it[:], in_=idxc2[:, g * CW2:(g + 1) * CW2])
                    if "gather" in ablate:
                        nc.sync.dma_start(
                            out=st[:].rearrange("p c d -> p (c d)"),
                            in_=junk2[:, :])
                    for c in range(nch2):
                        if "gather" in ablate:
                            break
                        tot = WG * q2[c]
                        for o in range(0, tot, 8):
                            n = min(8, tot - o)
                            nc.gpsimd.dma_gather(
                                out_ap=st[:, cb2[c] + o:cb2[c] + o + n, :],
                                in_ap=wh_tbl[b2[c]:b2[c + 1], :],
                                idxs_ap=it[:, (cb2[c] + o) * 8:(cb2[c] + o + n) * 8],
                                num_idxs=n * P,
                                num_idxs_reg=reg1024 if n == 8 else n * P,
                                elem_size=D,
                                queue_num=_next_q(),
                            )
                    rl = auxp.tile([P, ncols2], tdt, tag="rl2")
                    nc.sync.dma_start(out=rl[:], in_=rel2[:, g * ncols2:(g + 1) * ncols2])
                    mb = None
                    if "memb" not in ablate:
                        mb = membp.tile([P, ncols2, P], tdt, tag="mb2")
                        nc.vector.tensor_tensor(
                            out=mb[:],
                            in0=iota_t[:, None, :].to_broadcast([P, ncols2, P]),
                            in1=rl.to_broadcast([P, ncols2, P]),
                            op=mybir.AluOpType.is_equal,
                        )
                    if "nomm" in ablate:
                        continue
                    mb_src = st if "memb" in ablate else mb
                    tg = small.tile([P, WG, P], tdt, tag="tg")
                    for half in range(WG // 4):
                        accq = psum.tile([P, 4 * P], f32, space="PSUM",
                                         tag="acc", bufs=2)
                        for wq in range(4):
                            w = half * 4 + wq
                            cols = []
                            for c in range(nch2):
                                cols += [cb2[c] + w * q2[c] + t for t in range(q2[c])]
                            if "accmm" in ablate:
                                cols = cols[:1]
                            for i, col in enumerate(cols):
                                nc.tensor.matmul(
                                    out=accq[:, wq * P:(wq + 1) * P],
                                    lhsT=mb_src[:, col, 0:P], rhs=st[:, col, :],
                                    start=(i == 0), stop=(i == len(cols) - 1),
                                )
                        nc.scalar.copy(
                            out=tg[:, half * 4:(half + 1) * 4, :],
                            in_=accq[:].rearrange("p (w f) -> p w f", w=4))
                    row0 = g * WG * P
                    if "tblwrite" not in ablate:
                        nc.scalar.dma_start(
                            out=t_tbl[row0:row0 + WG * P, :].rearrange(
                                "(w p) f -> p w f", p=P),
                            in_=tg[:])
                    # RS/fin launches are delayed a few groups past their
                    # chunk's last t-write: the Pool (rs) / Act (fin) queues
                    # then see already-satisfied waits instead of draining
                    # the whole pipeline mid-phase.
                    if NG2 % RSC == 0:
                        if g >= RSD and (g - RSD + 1) % GPC2 == 0:
                            rs_start((g - RSD + 1) // GPC2 - 1)
                        if g >= RSD + FLG and (g - RSD - FLG + 1) % GPC2 == 0:
                            fin_chunk((g - RSD - FLG + 1) // GPC2 - 1)
                if "phase2" in ablate:
                    pass
                elif NG2 % RSC == 0:
                    for cidx in range(RSC):
                        if (cidx + 1) * GPC2 + RSD > NG2:
                            rs_start(cidx)
                    for cidx in range(RSC):
                        if (cidx + 1) * GPC2 + RSD + FLG > NG2:
                            fin_chunk(cidx)
                else:
                    for cidx in range(RSC):
                        rs_start(cidx)
                        fin_chunk(cidx)

    nc.compile()
    return nc


# ----------------------------------------------------------------------------
# SPMD runner (jitted shard_map over the 8 NeuronCores, cached for reuse)
# ----------------------------------------------------------------------------

class SpmdRunner:
    def __init__(self, nc, n_cores):
        import jax
        import concourse.mybir as mybir
        from concourse.bass2jax import (
            _bass_exec_p, install_neuronx_cc_hook, partition_id_tensor)
        from jax.sharding import Mesh, PartitionSpec
        from jax.experimental.shard_map import shard_map

        install_neuronx_cc_hook()
        self.jax = jax
        self.n_cores = n_cores
        partition_name = nc.partition_id_tensor.name if nc.partition_id_tensor else None
        in_names, out_names, out_avals, zero_shapes = [], [], [], []
        for alloc in nc.m.functions[0].allocations:
            if not isinstance(alloc, mybir.MemoryLocationSet):
                continue
            name = alloc.memorylocations[0].name
            if alloc.kind == "ExternalInput":
                if name != partition_name:
                    in_names.append(name)
            elif alloc.kind == "ExternalOutput":
                out_names.append(name)
                shape = tuple(alloc.tensor_shape)
                dtype = mybir.dt.np(alloc.dtype)
                out_avals.append(jax.core.ShapedArray(shape, dtype))
                zero_shapes.append((shape, dtype))
        self.in_names, self.out_names = in_names, out_names
        self.out_avals, self.zero_shapes = out_avals, zero_shapes
        n_params, n_outs = len(in_names), len(out_avals)
        all_in_names = list(in_names) + list(out_names)
        if partition_name is not None:
            all_in_names.append(partition_name)

        def _body(*args):
            operands = list(args)
            if partition_name is not None:
                operands.append(partition_id_tensor())
            outs = _bass_exec_p.bind(
                *operands,
                out_avals=tuple(out_avals),
                in_names=tuple(all_in_names),
                out_names=tuple(out_names),
                lowering_input_output_aliases=(),
                sim_require_finite=True,
                sim_require_nnan=True,
                nc=nc,
            )
            return tuple(outs)

        devices = jax.devices()[:n_cores]
        self.mesh = Mesh(np.asarray(devices), ("core",))
        in_specs = (PartitionSpec("core"),) * (n_params + n_outs)
        out_specs = (PartitionSpec("core"),) * n_outs
        self.fn = jax.jit(
            shard_map(_body, mesh=self.mesh, in_specs=in_specs,
                      out_specs=out_specs, check_rep=False),
            keep_unused=True,
        )
        self._device_args = None
        self._pspec = PartitionSpec

    def put_inputs(self, in_maps):
        jax = self.jax
        n = self.n_cores
        sharding = jax.sharding.NamedSharding(self.mesh, self._pspec("core"))
        args = []
        for name in self.in_names:
            cat = np.concatenate([np.asarray(in_maps[c][name]) for c in range(n)], axis=0)
            args.append(jax.device_put(cat, sharding))
        for shape, dtype in self.zero_shapes:
            z = np.zeros((n * shape[0], *shape[1:]), dtype)
            args.append(jax.device_put(z, sharding))
        self._device_args = args
        jax.block_until_ready(args)

    def run(self):
        outs = self.fn(*self._device_args)
        self.jax.block_until_ready(outs)
        return outs

    def results(self, outs):
        n = self.n_cores
        res = []
        for c in range(n):
            d = {}
            for i, name in enumerate(self.out_names):
                shp = self.out_avals[i].shape
                d[name] = np.asarray(outs[i]).reshape(n, *shp)[c]
            res.append(d)
        return res


# ----------------------------------------------------------------------------
# public entry point
# ----------------------------------------------------------------------------

_CACHE = {}


def _get_runner(meta, reps, ablate=()):
    key = (tuple(sorted(meta.items(), key=lambda kv: repr(kv[0]))).__repr__(), reps,
           tuple(sorted(ablate)))
    if key not in _CACHE:
        nc = _build_nc(meta, reps=reps, ablate=ablate)
        _CACHE[key] = SpmdRunner(nc, NCORES)
    return _CACHE[key]


def assemble(meta, res):
    """Reassemble per-core RS-chunked output slices into the full table."""
    RSC, TROWS, SLICE = meta["RSC"], meta["TROWS"], meta["SLICE"]
    CH, CHS = TROWS // RSC, SLICE // RSC
    full = np.empty((TROWS, D), np.float32)
    for c in range(RSC):
        for k in range(NCORES):
            full[c * CH + k * CHS:c * CH + (k + 1) * CHS] = \
                res[k]["out"][c * CHS:(c + 1) * CHS]
    return np.ascontiguousarray(full[:meta["n_lit"]])


def kernel(**inputs):
    meta, in_maps = _prep(inputs)
    r = _get_runner(meta, reps=1)
    r.put_inputs(in_maps)
    outs = r.run()
    res = r.results(outs)
    return assemble(meta, res)

